# revision 1
# baseline (speedup 1.0000x reference)
"""Trainium2 Bass kernel for a 2-layer GCN (GCNConv + ELU, x2), 8 NeuronCores.

Strategy (SPMD, graph/data parallel by dst node):
  Normalization factors as out = Ddst^-1/2 * (sum_src (Dsrc^-1/2 x)[src]) W + diag,
  so each conv AGGREGATES pre-scaled fp16 features first, then applies the
  dense weight matmul per 128-node dst tile, then a fused ELU epilogue
  (elu(y) = relu(y) - relu(1 - exp(y)) with the dinv scale folded into the
  scalar-engine activation args).

  Host-side work is index preprocessing only: degree counts, a degree-balanced
  node->slot permutation (equalizes per-tile edge counts so the static SPMD
  schedule pads minimally), edge sort by (dst tile group, src bucket, dst
  tile), and padding to a core-uniform chunk schedule. Self-loop terms are
  handled on-device as an identity matmul of each tile's own rows (no gather).

  Three device launches:
    A (prep, node-sharded): g1 = fp16(dinv * x)
    B (conv1): per dst tile: gather g1[src] rows (SWDGE dma_gather, 4 parallel
       queues, packed descriptors, int16 indices relative to 25088-row src
       buckets), scatter-add via one-hot matmuls (S = is_equal(iota, dstloc)
       built on the vector engine, one tensor_tensor per gather call),
       + diagonal identity matmul, @W1, ELU; emits fp16(dinv * out1).
    C (conv2): same with W2; emits fp32 out2. Host un-permutes rows.
"""
"""GCN kernel builder for TRN2: schedule construction (host) + bass kernel builders."""
import dataclasses
import numpy as np
import concourse.bacc as bacc
import concourse.mybir as mybir
import concourse.tile as tile

P = 128
N_CORES = 8


def build_schedule(edge_index, n_nodes, tiles_per_core, group_tiles, bucket_rows):
    """Build the static per-core gather/scatter schedule.

    Edges (+self-loops) are assigned to the core owning their dst tile.
    Per core, edges are ordered by (group, src_bucket, tile); each
    (tile, bucket) segment is padded to a whole number of 128-edge chunks,
    and chunk counts are made uniform across cores (max), padding with
    dummy edges (src=0, dstloc=sentinel 300 -> zero contribution).

    Returns dict with per-core arrays and the uniform chunk schedule.
    """
    src = np.asarray(edge_index[0], dtype=np.int64).astype(np.int32)
    dst = np.asarray(edge_index[1], dtype=np.int64).astype(np.int32)
    # self-loops count toward degree but are handled as a diagonal add in the
    # kernel (no gather), not as edges
    deg = (np.bincount(dst, minlength=n_nodes) + 1).astype(np.float64)
    dinv = np.where(deg > 0, 1.0 / np.sqrt(deg), 0.0).astype(np.float32)

    n_tiles_total = tiles_per_core * N_CORES
    n_slots = n_tiles_total * P
    assert n_slots >= n_nodes

    # Degree-balanced node->slot permutation: snake-deal nodes (sorted by
    # in-degree) across tiles so every dst tile gets a near-equal edge count.
    # This minimizes the cross-core max of per-(tile, bucket) chunk counts,
    # which sets the static (padded) schedule size.
    order_by_deg = np.argsort(-deg, kind="stable")
    slot_of = np.empty(n_nodes, dtype=np.int64)
    fill = np.zeros(n_tiles_total, dtype=np.int64)
    pos = 0
    fwd = True
    for i in range(0, n_nodes, n_tiles_total):
        blk = order_by_deg[i:i + n_tiles_total]
        k = len(blk)
        tiles_order = np.arange(n_tiles_total) if fwd else np.arange(n_tiles_total)[::-1]
        tsel = tiles_order[:k]
        slot_of[blk] = tsel * P + fill[tsel]
        fill[tsel] += 1
        fwd = not fwd
    assert fill.max() <= P
    src = slot_of[src].astype(np.int32)
    dst = slot_of[dst].astype(np.int32)
    dinv_slot = np.zeros(n_slots, dtype=np.float32)
    dinv_slot[slot_of] = dinv
    dinv = dinv_slot
    n_nodes = n_slots
    n_buckets = (n_nodes + bucket_rows - 1) // bucket_rows

    tile_of = dst >> 7
    core_of = tile_of // tiles_per_core
    bucket_of = src // bucket_rows

    n_groups = (tiles_per_core + group_tiles - 1) // group_tiles

    # count chunks per (core, tile_local, bucket)
    counts = np.zeros((N_CORES, tiles_per_core, n_buckets), dtype=np.int64)
    # sort edges by (core, tile_local, bucket) once
    order = np.lexsort((bucket_of, tile_of))
    s_src, s_dst, s_tile, s_bucket = src[order], dst[order], tile_of[order], bucket_of[order]
    np.add.at(counts, (s_tile // tiles_per_core, s_tile % tiles_per_core, s_bucket), 1)

    nchunk = (counts + P - 1) // P  # chunks per (core, t, b)
    nchunk_u = nchunk.max(axis=0)   # uniform over cores [tiles_per_core, n_buckets]
    # ensure every tile has >=1 chunk in bucket 0 (so psum gets written)
    for t in range(tiles_per_core):
        if nchunk_u[t].sum() == 0:
            nchunk_u[t][0] = 1

    # chunk sequence (uniform): ordered by (group, bucket, tile_local)
    chunk_tile = []   # tile_local of each chunk
    chunk_gb = []     # (group, bucket) of each chunk
    gb_nchunks = np.zeros((n_groups, n_buckets), dtype=np.int64)
    for g in range(n_groups):
        t0, t1 = g * group_tiles, min((g + 1) * group_tiles, tiles_per_core)
        for b in range(n_buckets):
            for t in range(t0, t1):
                for _ in range(int(nchunk_u[t, b])):
                    chunk_tile.append(t)
                    chunk_gb.append((g, b))
            gb_nchunks[g, b] = sum(int(nchunk_u[t, b]) for t in range(t0, t1))
    n_chunks_total = len(chunk_tile)
    chunk_tile = np.array(chunk_tile, dtype=np.int32)

    # per-core edge placement into the uniform chunk layout
    # slot base for each (t,b) in the chunk stream:
    slot_base = {}
    pos = 0
    for g in range(n_groups):
        t0, t1 = g * group_tiles, min((g + 1) * group_tiles, tiles_per_core)
        for b in range(n_buckets):
            for t in range(t0, t1):
                slot_base[(t, b)] = pos
                pos += int(nchunk_u[t, b]) * P
    assert pos == n_chunks_total * P

    idx_rel = np.zeros((N_CORES, n_chunks_total * P), dtype=np.int16)
    dstloc = np.full((N_CORES, n_chunks_total * P), 300.0, dtype=np.float16)

    # place each core's real edges
    for c in range(N_CORES):
        m = (s_tile // tiles_per_core) == c
        c_src, c_dst = s_src[m], s_dst[m]
        c_t, c_b = (s_tile[m] % tiles_per_core), s_bucket[m]
        # edges already sorted by (tile, bucket); offset within segment:
        # compute running position within each (t,b)
        key = c_t.astype(np.int64) * n_buckets + c_b
        # positions within each key-run (data sorted by key)
        startd = np.r_[True, key[1:] != key[:-1]]
        run_id = np.cumsum(startd) - 1
        run_start = np.nonzero(startd)[0]
        within = np.arange(len(key)) - run_start[run_id]
        base = np.array([slot_base[(int(t), int(b))] for t, b in zip(c_t[startd], c_b[startd])])
        gpos = base[run_id] + within
        idx_rel[c, gpos] = (c_src - c_b * bucket_rows).astype(np.int16)
        dstloc[c, gpos] = (c_dst & (P - 1)).astype(np.float16)

    # wrap idxs: idx i of call -> [16 partitions, i//16], replicated 8x
    # calls are per (g,b): contiguous span of gb_nchunks[g,b]*128 idxs
    idx_cols_total = n_chunks_total * P // 16
    idx_wrapped = np.zeros((N_CORES, P, idx_cols_total), dtype=np.int16)
    col_off = 0
    gb_meta = []  # (g, b, chunk_start, nch, idx_col_start)
    cpos = 0
    for g in range(n_groups):
        for b in range(n_buckets):
            nch = int(gb_nchunks[g, b])
            ni = nch * P
            if nch == 0:
                gb_meta.append((g, b, cpos, 0, col_off))
                continue
            span = slice(cpos * P, cpos * P + ni)
            blk = idx_rel[:, span].reshape(N_CORES, ni // 16, 16)
            w = np.transpose(blk, (0, 2, 1))  # [cores, 16, cols]
            idx_wrapped[:, :, col_off:col_off + ni // 16] = np.tile(w, (1, 8, 1))
            gb_meta.append((g, b, cpos, nch, col_off))
            col_off += ni // 16
            cpos += nch
    assert cpos == n_chunks_total

    # dstloc transposed: [cores, 128, n_chunks_total]; column ch = dstloc of edges ch*128..+128
    dstloc_T = np.transpose(dstloc.reshape(N_CORES, n_chunks_total, P), (0, 2, 1)).copy()

    # per-core dinv (padded to tiles_per_core*128), transposed [128, tiles_per_core]
    dinv_T = dinv.reshape(N_CORES, tiles_per_core, P).transpose(0, 2, 1).copy()

    iota = np.tile(np.arange(P, dtype=np.float16)[None, :], (P, 1))

    return dict(
        dinv=dinv, dinv_T=dinv_T, iota=iota, slot_of=slot_of,
        idx_wrapped=idx_wrapped, dstloc_T=dstloc_T,
        chunk_tile=chunk_tile, gb_meta=gb_meta, gb_nchunks=gb_nchunks,
        n_groups=n_groups, n_buckets=n_buckets, n_chunks_total=n_chunks_total,
        tiles_per_core=tiles_per_core, group_tiles=group_tiles,
        bucket_rows=bucket_rows, n_nodes=n_nodes,
    )


def build_prep_kernel(n_nodes_pad_core, feat, R=1):
    """g_shard = fp16(dinv * x_shard). n_nodes_pad_core multiple of 128."""
    nc = bacc.Bacc("TRN2")
    nt = n_nodes_pad_core // P
    x = nc.dram_tensor("x", [n_nodes_pad_core, feat], mybir.dt.float32, kind="ExternalInput")
    dinvT = nc.dram_tensor("dinvT", [P, nt], mybir.dt.float32, kind="ExternalInput")
    g = nc.dram_tensor("g", [n_nodes_pad_core, feat], mybir.dt.float16, kind="ExternalOutput")
    with tile.TileContext(nc) as tc:
        with tc.tile_pool(name="sb", bufs=8) as pool, \
             tc.tile_pool(name="cst", bufs=1) as cpool:
            dv = cpool.tile([P, nt], mybir.dt.float32)
            nc.sync.dma_start(dv[:], dinvT[:])
            for _ in range(R):
                for t in range(nt):
                    xt = pool.tile([P, feat], mybir.dt.float32, tag="x")
                    nc.sync.dma_start(xt[:], x[t*P:(t+1)*P, :])
                    gt = pool.tile([P, feat], mybir.dt.float16, tag="g")
                    nc.vector.tensor_scalar(gt[:], xt[:], dv[:, t:t+1], None, mybir.AluOpType.mult)
                    nc.sync.dma_start(g[t*P:(t+1)*P, :], gt[:])
    nc.finalize()
    return nc


def build_conv_kernel(sched, feat_in, feat_out, out_fp16_scaled, R=1, no_diag=False,
                      msg_bufs=12, s_bufs=5, ep_bufs=6, tp_bufs=6, w_dtype=mybir.dt.float16,
                      max_groups=None):
    """One GCN conv layer (aggregate-first).

    inputs: g [n_src_rows, feat_in] fp16, W [feat_in, feat_out] fp16,
            dinvT [128, tiles_per_core] fp32, iota [128,128] fp16,
            idxs [128, idx_cols] int16, dstlocT [128, n_chunks] fp16
    output: out [tiles_per_core*128, feat_out] (fp16 scaled by dinv, or fp32 plain)
    """
    tpc = sched["tiles_per_core"]
    gt = sched["group_tiles"]
    n_groups = sched["n_groups"]
    n_buckets = sched["n_buckets"]
    nct = sched["n_chunks_total"]
    chunk_tile = sched["chunk_tile"]
    gb_meta = sched["gb_meta"]
    bucket_rows = sched["bucket_rows"]
    n_nodes = sched["n_nodes"]
    idx_cols = sched["idx_wrapped"].shape[2]

    # first/last chunk per tile for psum start/stop flags
    first_chunk = {}
    last_chunk = {}
    for ci, t in enumerate(chunk_tile):
        t = int(t)
        if t not in first_chunk:
            first_chunk[t] = ci
        last_chunk[t] = ci

    out_dtype = mybir.dt.float16 if out_fp16_scaled else mybir.dt.float32

    nc = bacc.Bacc("TRN2", num_swdge_queues=4)
    g = nc.dram_tensor("g", [n_nodes, feat_in], mybir.dt.float16, kind="ExternalInput")
    gown = nc.dram_tensor("gown", [tpc * P, feat_in], mybir.dt.float16, kind="ExternalInput")
    W = nc.dram_tensor("W", [feat_in, feat_out], w_dtype, kind="ExternalInput")
    dinvT = nc.dram_tensor("dinvT", [P, tpc], mybir.dt.float32, kind="ExternalInput")
    iota_t = nc.dram_tensor("iota", [P, P], mybir.dt.float16, kind="ExternalInput")
    ident_t = nc.dram_tensor("ident", [P, P], mybir.dt.float16, kind="ExternalInput")
    idxs = nc.dram_tensor("idxs", [P, idx_cols], mybir.dt.int16, kind="ExternalInput")
    dstlocT = nc.dram_tensor("dstlocT", [P, nct], mybir.dt.float16, kind="ExternalInput")
    out = nc.dram_tensor("out", [tpc * P, feat_out], out_dtype, kind="ExternalOutput")

    with tile.TileContext(nc) as tc:
        with tc.tile_pool(name="cst", bufs=1) as cpool, \
             tc.tile_pool(name="msg", bufs=msg_bufs) as mpool, \
             tc.tile_pool(name="sS", bufs=s_bufs) as spool, \
             tc.tile_pool(name="agg", bufs=1, space="PSUM") as apool, \
             tc.tile_pool(name="ops", bufs=2, space="PSUM") as opool, \
             tc.tile_pool(name="eps", bufs=ep_bufs) as epool, \
             tc.tile_pool(name="outp", bufs=tp_bufs) as tpool:
            # constants
            w_sb = cpool.tile([feat_in, feat_out], w_dtype)
            nc.sync.dma_start(w_sb[:], W[:])
            dv = cpool.tile([P, tpc], mybir.dt.float32)
            nc.sync.dma_start(dv[:], dinvT[:])
            io = cpool.tile([P, P], mybir.dt.float16)
            nc.sync.dma_start(io[:], iota_t[:])
            idn = cpool.tile([P, P], mybir.dt.float16)
            nc.sync.dma_start(idn[:], ident_t[:])
            ix = cpool.tile([P, idx_cols], mybir.dt.int16)
            nc.sync.dma_start(ix[:], idxs[:])
            dl = cpool.tile([P, nct], mybir.dt.float16)
            nc.sync.dma_start(dl[:], dstlocT[:])

            for _ in range(R):
                for gi in range(n_groups if max_groups is None else min(max_groups, n_groups)):
                    t0 = gi * gt
                    t1 = min((gi + 1) * gt, tpc)
                    ntg = t1 - t0
                    nbanks = ntg  # one PSUM bank per tile (has_written is bank-granular)
                    # aggT psum: nbanks tensors of [128, 512]
                    banks = [apool.tile([P, 512], mybir.dt.float32, tag=f"agg{k}",
                                        name=f"aggb_{gi}_{k}")
                             for k in range(nbanks)]

                    def agg_slice(t):
                        return banks[t - t0][:, :P]

                    # diagonal (self-loop) term: aggT[:, d] += g[tile rows d]^T
                    # via PE transpose; first writer of each psum slice.
                    for t in range(t0, t1):
                        gd = tpool.tile([P, feat_in], mybir.dt.float16, tag="gdiag")
                        nc.sync.dma_start(gd[:], gown[t*P:(t+1)*P, :])
                        nc.tensor.matmul(agg_slice(t), lhsT=gd[:], rhs=idn[:],
                                         start=True, stop=no_diag)

                    for b in range(n_buckets):
                        meta = gb_meta[gi * n_buckets + b]
                        _, _, c_start, nch, col0 = meta
                        if nch == 0:
                            continue
                        msg = mpool.tile([P, nch, feat_in], mybir.dt.float16, tag="msg")
                        base = b * bucket_rows
                        rows = min(bucket_rows, n_nodes - base)
                        nc.gpsimd.dma_gather(
                            msg[:], g[base:base + rows, :],
                            ix[:, col0:col0 + nch * P // 16],
                            nch * P, nch * P, feat_in,
                            single_packet=False, queue_num=(gi * n_buckets + b) % 4,
                        )
                        S = spool.tile([P, nch, P], mybir.dt.float16, tag="S")
                        dl_b = dl[:, c_start:c_start + nch].to_broadcast([P, nch, P])
                        io_ap = io[:, :]
                        io_rep = dataclasses.replace(
                            io_ap, ap=[io_ap.ap[0], [0, nch], io_ap.ap[1]])
                        nc.vector.tensor_tensor(
                            S[:], io_rep, dl_b, mybir.AluOpType.is_equal)
                        for k in range(nch):
                            ci = c_start + k
                            t = int(chunk_tile[ci])
                            nc.tensor.matmul(
                                agg_slice(t), lhsT=msg[:, k, :], rhs=S[:, k, :],
                                start=False,
                                stop=(ci == last_chunk[t]))
                    # finalize tiles of this group
                    for t in range(t0, t1):
                        aggsb = tpool.tile([P, P], mybir.dt.float16, tag="aggsb")
                        nc.vector.tensor_copy(aggsb[:], agg_slice(t))
                        ops = opool.tile([P, feat_out], mybir.dt.float32, tag="ops")
                        nc.tensor.matmul(ops[:], lhsT=aggsb[:], rhs=w_sb[:],
                                         start=True, stop=True)
                        dvt = dv[:, t:t+1]
                        e = epool.tile([P, feat_out], mybir.dt.float32, tag="e")
                        nc.scalar.activation(e[:], ops[:],
                                             mybir.ActivationFunctionType.Exp,
                                             scale=dvt)
                        r = epool.tile([P, feat_out], mybir.dt.float32, tag="r")
                        nc.scalar.activation(r[:], e[:],
                                             mybir.ActivationFunctionType.Relu,
                                             bias=1.0, scale=-1.0)
                        p = epool.tile([P, feat_out], mybir.dt.float32, tag="p")
                        nc.scalar.activation(p[:], ops[:],
                                             mybir.ActivationFunctionType.Relu,
                                             scale=dvt)
                        ot = tpool.tile([P, feat_out], out_dtype, tag="ot")
                        if out_fp16_scaled:
                            elu = epool.tile([P, feat_out], mybir.dt.float32, tag="elu")
                            nc.vector.tensor_tensor(elu[:], p[:], r[:],
                                                    mybir.AluOpType.subtract)
                            nc.vector.tensor_scalar(ot[:], elu[:], dvt, None,
                                                    mybir.AluOpType.mult)
                        else:
                            nc.vector.tensor_tensor(ot[:], p[:], r[:],
                                                    mybir.AluOpType.subtract)
                        nc.sync.dma_start(out[t*P:(t+1)*P, :], ot[:])
    nc.finalize()
    return nc

import sys as _sys
import types as _types


def _ensure_axon_stub():
    """run_bass_kernel_spmd(trace=True) under axon imports antenv.axon_hooks;
    provide a no-op stub when the module is absent in this container."""
    try:
        import antenv.axon_hooks  # noqa
    except ModuleNotFoundError:
        try:
            import antenv
        except ModuleNotFoundError:
            antenv = _types.ModuleType("antenv")
            _sys.modules["antenv"] = antenv
        import antenv
        m = _types.ModuleType("antenv.axon_hooks")
        m.get_axon_ntff_profile_hook = lambda: None
        _sys.modules["antenv.axon_hooks"] = m
        antenv.axon_hooks = m


N_NODES = 100000
TPC = 98          # dst tiles per core
GT = 6            # dst tiles per PSUM group
BROWS = 25088     # src bucket rows (int16 gather index range; 100352/4)
IN_DIM, HID_DIM, OUT_DIM = 128, 128, 64


def kernel(x, edge_index, W1, b1, W2, b2):
    _ensure_axon_stub()
    from concourse.bass_utils import run_bass_kernel_spmd

    x = np.asarray(x, dtype=np.float32)
    edge_index = np.asarray(edge_index)
    W1 = np.asarray(W1, dtype=np.float32)
    W2 = np.asarray(W2, dtype=np.float32)
    b1 = np.asarray(b1, dtype=np.float32)
    b2 = np.asarray(b2, dtype=np.float32)
    assert np.all(b1 == 0) and np.all(b2 == 0), "kernel assumes zero conv biases"

    sched = build_schedule(edge_index, N_NODES, TPC, GT, BROWS)
    slot_of = sched["slot_of"]
    ident = np.eye(P, dtype=np.float16)
    rows_pc = TPC * P
    cores = list(range(N_CORES))

    # ---- launch A: g1 = fp16(dinv * x), node-sharded (slot space)
    x_pad = np.zeros((rows_pc * N_CORES, IN_DIM), np.float32)
    x_pad[slot_of] = x
    nc_a = build_prep_kernel(rows_pc, IN_DIM)
    in_a = [{"x": x_pad[c*rows_pc:(c+1)*rows_pc], "dinvT": sched["dinv_T"][c]}
            for c in cores]
    res_a = run_bass_kernel_spmd(nc_a, in_a, core_ids=cores, trace=False)
    g1 = np.concatenate([res_a.results[c]["g"] for c in cores])

    common = lambda c: {"dinvT": sched["dinv_T"][c], "iota": sched["iota"],
                        "ident": ident, "idxs": sched["idx_wrapped"][c],
                        "dstlocT": sched["dstloc_T"][c]}

    # ---- launch B: conv1 -> fp16(dinv * elu(.))
    nc_b = build_conv_kernel(sched, IN_DIM, HID_DIM, out_fp16_scaled=True)
    in_b = [{"g": g1, "gown": g1[c*rows_pc:(c+1)*rows_pc],
             "W": W1.astype(np.float16), **common(c)} for c in cores]
    res_b = run_bass_kernel_spmd(nc_b, in_b, core_ids=cores, trace=False)
    g2 = np.concatenate([res_b.results[c]["out"] for c in cores])

    # ---- launch C: conv2 -> fp32 elu(.)
    nc_c = build_conv_kernel(sched, HID_DIM, OUT_DIM, out_fp16_scaled=False)
    in_c = [{"g": g2, "gown": g2[c*rows_pc:(c+1)*rows_pc],
             "W": W2.astype(np.float16), **common(c)} for c in cores]
    res_c = run_bass_kernel_spmd(nc_c, in_c, core_ids=cores, trace=False)
    out = np.concatenate([res_c.results[c]["out"] for c in cores])[slot_of]
    return np.ascontiguousarray(out.astype(np.float32))

